# revision 22
# baseline (speedup 1.0000x reference)
"""Trainium2 Bass kernel for nn_BDHBlock (pre-LN latent block with
softmax-free attention and sigmoid gating).

Sharding: data-parallel over batch B=16 across 8 cores (2 per core).
No collectives; outputs are gathered/re-laid-out on the host.

Per-core math (B_loc=2, N=1024, D=768, H=12, HD=64), all matmuls fp16
with fp32 PSUM accumulation:
  z    = (x - mu) * rstd                          (token-major)
  lat  = relu(z @ enc_w'.T + enc_b')              (feature-major)
  qk   = rope(lat @ qk_w.T + qk_b) / sqrt(sqrt(HD))   (token-major)
  v    = lat @ v_w.T + v_b                        (token-major)
  T_h  = qk_h^T @ v_h         per (b,h)           [HD, HD]
  attn_h = qk_h @ T_h      (== (qk qk^T/8) v by associativity)
  gated = sigmoid(z @ gate_w'.T + gate_b') * (attn @ out_w.T + out_b)
  out  = x + gated

enc_w' = enc_w*diag(ln_w), enc_b' = enc_b + enc_w@ln_b (same for gate)
fold the LayerNorm affine into the weights host-side.  DMA on this
part is latency-bound per instruction (~4us for 128 partition lines
regardless of bytes), so x ships fp16 in a per-partition-contiguous
host layout and loads in 3 instructions into a resident slab that
also serves the residual (no reload); the output stores fp16 in the
same layout, 2 tiles per instruction, and the host converts back.
Weights ride the gpsimd SWDGE ring.  LayerNorm emission is software-
pipelined (stats of tile i ahead of the apply of tile i-1) so the
in-order DVE queue never starves the per-tile scalar chain.  Rope is
o = x*A + swapped_halves(x*C) with host-precomputed fp16 tables (sign
folded into C), emitted one batch behind the qk drains so it never
blocks them.  xn^T and gated^T transposes run on PE; qk^T uses the
DMA XBAR (its consumers are far downstream).  Gate/out projections
are feature-major so biases ride the ACT drain (sigmoid fused), and
the residual add reads the PE-transposed gated result straight from
PSUM.  The softmax-free attention makes scores@v associative, so the
N x N score matrices are never materialized.
"""

import os
import sys

for _p in ("/opt/trn_rl_repo", "/root/.axon_site/_ro/trn_rl_repo"):
    if os.path.isdir(_p) and _p not in sys.path:
        sys.path.insert(0, _p)

import math
import numpy as np

import concourse.bass as bass
import concourse.mybir as mybir
from concourse import bacc
from concourse import bass_utils
from concourse.bass import ts, ds
from concourse.tile import TileContext
from concourse.masks import make_identity

F32 = mybir.dt.float32
F16 = mybir.dt.float16
AF = mybir.ActivationFunctionType

P = 128          # partitions
D = 768
KT = D // P      # 6 d-tiles
B_LOC = 2        # batch elements per core
SEQ = 1024
T = B_LOC * SEQ  # 2048 tokens per core
NT = T // P      # 16 token tiles
TPB = SEQ // P   # 8 token tiles per batch element
TW = 512         # token window (feature-major matmul free dim)
NTW = T // TW    # 4
JW = 384         # feature window (token-major matmul free dim)
NJW = D // JW    # 2
H = 12
HD = 64
EPS = 1e-5
QK_SCALE = 1.0 / math.sqrt(math.sqrt(HD))  # applied twice => 1/sqrt(HD)
RB = 4           # token tiles per rope batch / transpose window
SB = 2           # token tiles per output store

W_NAMES = ["enc_w", "qk_w", "v_w", "out_w", "gate_w"]
BV_QK, BV_V = 0, 1               # bvec rows (broadcast free-dim biases)
PB_ENC, PB_OUT, PB_GATE = 0, 1, 2  # pbias rows (per-partition biases)


def build_nc():
    nc = bacc.Bacc("TRN2", target_bir_lowering=False, debug=False)

    x_in = nc.dram_tensor("x", [P, NT * D], F16, kind="ExternalInput")
    ac_in = nc.dram_tensor("rope_ac", [P, 2, TPB, D], F16,
                           kind="ExternalInput")
    pbias_in = nc.dram_tensor("pbias", [P, 3, KT], F32, kind="ExternalInput")
    bvec_in = nc.dram_tensor("bvecs", [P, 2, D], F16, kind="ExternalInput")
    w_in = {nm: nc.dram_tensor(nm, [P, KT * D], F16, kind="ExternalInput")
            for nm in W_NAMES}
    out_t = nc.dram_tensor("out", [P, NT * D], F16, kind="ExternalOutput")

    with TileContext(nc) as tc:
        with (
            tc.tile_pool(name="consts", bufs=1) as cp,
            tc.tile_pool(name="wrot", bufs=3) as wrot,
            tc.tile_pool(name="big", bufs=4) as bigp,
            tc.tile_pool(name="xslab", bufs=1) as xsp,
            tc.tile_pool(name="work", bufs=2) as wk,
            tc.tile_pool(name="stats", bufs=4) as stp,
            tc.tile_pool(name="ropem2", bufs=1) as rm2,
            tc.tile_pool(name="qraw", bufs=2) as qrp,
            tc.tile_pool(name="gwk", bufs=2) as gwk,
            tc.tile_pool(name="xo", bufs=2) as xop,
            tc.tile_pool(name="tbuf", bufs=12) as tbp,
            tc.tile_pool(name="psA", bufs=3, space="PSUM") as psA,
            tc.tile_pool(name="psB", bufs=3, space="PSUM") as psB,
            tc.tile_pool(name="psT", bufs=2, space="PSUM") as psT,
        ):
            # ------------- constants / weights -----------------------
            # rope A/C tables take a big-pool ring slot; they are dead
            # after the last rope batch and vtm reuses the slot.
            ac = bigp.tile([P, 2, TPB, D], F16, tag="big", name="rope_ac")
            with nc.named_scope("prep"):
                eps_t = cp.tile([P, 1], F32, tag="epsc")
                nc.vector.memset(eps_t[:], EPS)
                ident = cp.tile([P, P], F16, tag="ident")
                make_identity(nc, ident[:])
                # per-partition biases for feature-major drains
                pbias = cp.tile([P, 3, KT], F32, tag="pbias")
                nc.gpsimd.dma_start(pbias[:], pbias_in.ap())

            xnT = cp.tile([P, KT, T], F16, tag="xnT")

            # x: resident slab, loaded in 3 latency-bound instructions
            # on the sync ring (first covers tiles 0-1 so LN starts asap)
            xs = xsp.tile([P, NT, D], F16, tag="xs")
            for lo, hi in ((0, 2), (2, 4), (4, 8), (8, NT)):
                nc.sync.dma_start(xs[:, lo:hi, :],
                                  x_in.ap()[:, ds(lo * D, (hi - lo) * D)])

            # weights / tables on the gpsimd SWDGE ring (own ring, and
            # DMA here is latency- not bandwidth-bound)
            wT = {}
            wT["enc_w"] = wrot.tile([P, KT, D], F16, tag="wT", name="wT_enc")
            nc.gpsimd.dma_start(wT["enc_w"][:], w_in["enc_w"].ap())
            bvec = cp.tile([P, 2, D], F16, tag="bvec")
            nc.gpsimd.dma_start(bvec[:], bvec_in.ap())
            nc.gpsimd.dma_start(ac[:], ac_in.ap())
            for nm in ["qk_w", "v_w"]:
                wT[nm] = wrot.tile([P, KT, D], F16, tag="wT", name=f"wT_{nm}")
                nc.gpsimd.dma_start(wT[nm][:], w_in[nm].ap())

            # ---------------- LayerNorm (token-major) ----------------
            # software-pipelined: stats of tile i are emitted ahead of
            # the apply of tile i-1 so the in-order DVE queue never
            # waits on a not-yet-landed tile before finishing an apply
            def ln_stats(i):
                xg = xs[:, i, :].rearrange("p (s c) -> p s c", c=256)
                stats = stp.tile([P, 3, 6], F32, tag="bnstats")
                for s in range(3):
                    nc.vector.bn_stats(stats[:, s, :], xg[:, s, :])
                mv = stp.tile([P, 2], F32, tag="bnmv")
                nc.vector.bn_aggr(mv[:], stats[:])
                return mv

            def ln_apply(i, mv):
                rs = stp.tile([P, 1], F32, tag="rstd")
                nc.scalar.activation(rs[:], mv[:, 1:2], AF.Sqrt,
                                     bias=eps_t[:])
                nc.vector.reciprocal(rs[:], rs[:])
                nb = stp.tile([P, 1], F32, tag="negmurs")
                nc.vector.tensor_scalar(
                    nb[:], mv[:, 0:1], rs[:], -1.0,
                    op0=mybir.AluOpType.mult, op1=mybir.AluOpType.mult)
                xn16 = wk.tile([P, D], F16, tag="xn16")
                nc.scalar.activation(xn16[:], xs[:, i, :], AF.Identity,
                                     bias=nb[:], scale=rs[:])
                # feature-major via PE transposes (PE is idle here),
                # batched into one PSUM bank + one DVE drain
                pt = psT.tile([P, D], F16, tag="psT")
                for k in range(KT):
                    nc.tensor.transpose(pt[:, ts(k, P)], xn16[:, ts(k, P)],
                                        ident[:])
                nc.vector.tensor_copy(
                    xnT[:, :, ts(i, P)],
                    pt[:].rearrange("p (k c) -> p k c", c=P))

            with nc.named_scope("ln"):
                mvs = {}
                mvs[0] = ln_stats(0)
                for i in range(NT):
                    if i + 1 < NT:
                        mvs[i + 1] = ln_stats(i + 1)
                    ln_apply(i, mvs.pop(i))

            # ---------------- encoder: latT = relu(Wenc @ xn^T) ------
            latT = bigp.tile([P, KT, T], F16, tag="big", name="latT")
            with nc.named_scope("enc"):
                for tw in range(NTW):
                    for j in range(KT):
                        ps = psA.tile([P, TW], F32, tag="psA")
                        for k in range(KT):
                            nc.tensor.matmul(
                                ps[:], wT["enc_w"][:, k, ts(j, P)],
                                xnT[:, k, ts(tw, TW)],
                                start=(k == 0), stop=(k == KT - 1))
                        nc.vector.tensor_scalar(
                            latT[:, j, ts(tw, TW)], ps[:],
                            pbias[:, PB_ENC, j:j + 1], 0.0,
                            op0=mybir.AluOpType.add,
                            op1=mybir.AluOpType.max)

            # late weights reuse the first two wrot slots (deps auto-wait)
            for nm in ["out_w", "gate_w"]:
                wT[nm] = wrot.tile([P, KT, D], F16, tag="wT", name=f"wT_{nm}")
                nc.gpsimd.dma_start(wT[nm][:], w_in[nm].ap())

            # ---------------- qk (token-major) + rope ----------------
            qkR = bigp.tile([P, NT, D], F16, tag="big", name="qkR")
            qkT = bigp.tile([P, KT, T], F16, tag="big", name="qkT")
            qraws = {}

            def qk_mm(g):
                qraw = qrp.tile([P, RB, D], F16, tag="qraw")
                qraws[g] = qraw
                for r in range(RB):
                    i = g * RB + r
                    for jw in range(NJW):
                        ps = psB.tile([P, JW], F32, tag="psB")
                        for k in range(KT):
                            nc.tensor.matmul(
                                ps[:], latT[:, k, ts(i, P)],
                                wT["qk_w"][:, k, ts(jw, JW)],
                                start=(k == 0), stop=(k == KT - 1))
                        nc.vector.tensor_add(
                            qraw[:, r, ts(jw, JW)], ps[:],
                            bvec[:, BV_QK, ts(jw, JW)])

            def rope_muls(g, m2_engine):
                # o = x*A + swap_halves(x*C); m2 first (m1 is in-place)
                qraw = qraws[g]
                ti0 = (g * RB) % TPB
                aA = ac[:, 0, ds(ti0, RB), :]
                aC = ac[:, 1, ds(ti0, RB), :]
                m2 = rm2.tile([P, RB, D], F16, tag="ropem2")
                m2_engine.tensor_mul(m2[:], qraw[:], aC)
                nc.vector.tensor_mul(qraw[:], qraw[:], aA)
                return m2

            def rope_adds(g, m2):
                # adds recombine halves across m1/m2
                qraw = qraws.pop(g)
                m1h = qraw[:].rearrange("p t (f d) -> p t f d", d=HD)
                m2h = m2[:].rearrange("p t (f d) -> p t f d", d=HD)
                oh = qkR[:, ds(g * RB, RB), :].rearrange(
                    "p t (f d) -> p t f d", d=HD)
                nc.vector.tensor_add(
                    oh[:, :, :, 0:HD // 2],
                    m1h[:, :, :, 0:HD // 2], m2h[:, :, :, HD // 2:])
                nc.vector.tensor_add(
                    oh[:, :, :, HD // 2:],
                    m1h[:, :, :, HD // 2:], m2h[:, :, :, 0:HD // 2])
                # feature-major copy via DMA XBAR transpose
                for r in range(RB):
                    i = g * RB + r
                    nc.sync.dma_start(qkT[:, :, ts(i, P)], qkR[:, i, :],
                                      transpose=True)

            def rope(g, m2_engine=None):
                rope_adds(g, rope_muls(g, m2_engine or nc.vector))

            with nc.named_scope("qk"):
                for g in range(NT // RB):
                    qk_mm(g)
                    if g >= 1:
                        rope(g - 1)  # one batch behind: never blocks drains

            # ---------------- v (token-major) ------------------------
            vtm = bigp.tile([P, NT, D], F16, tag="big", name="v")
            with nc.named_scope("v"):
                for i in range(NT):
                    for jw in range(NJW):
                        ps = psB.tile([P, JW], F32, tag="psB")
                        for k in range(KT):
                            nc.tensor.matmul(
                                ps[:], latT[:, k, ts(i, P)],
                                wT["v_w"][:, k, ts(jw, JW)],
                                start=(k == 0), stop=(k == KT - 1))
                        nc.vector.tensor_add(vtm[:, i, ts(jw, JW)], ps[:],
                                             bvec[:, BV_V, ts(jw, JW)])
                    if i == 1:
                        rope(NT // RB - 1)

            # ---------------- attention ------------------------------
            # M1: T_h = qk_h^T @ v_h  [HD, HD] per (b, head); head pairs
            # packed into array column halves.  M2: attnT_h = T_h^T @ qkT_h.
            t16s = {}
            with nc.named_scope("attn_m1"):
                for b in range(B_LOC):
                    for hp in range(KT):
                        hA, hB = 2 * hp, 2 * hp + 1
                        # the two concurrent accum groups live on disjoint
                        # partition ranges / array quadrants; the sim's
                        # bank-granular group check is stricter than HW
                        pt = psB.tile([P, HD], F32, tag="psB",
                                      name=f"ptm1_{b}_{hp}")
                        for m in range(TPB):
                            mt = b * TPB + m
                            nc.tensor.matmul(
                                pt[0:HD, :],
                                qkR[:, mt, ts(hA, HD)], vtm[:, mt, ts(hA, HD)],
                                start=(m == 0), stop=(m == TPB - 1),
                                tile_position=(0, 0), skip_group_check=True)
                            nc.tensor.matmul(
                                pt[HD:P, :],
                                qkR[:, mt, ts(hB, HD)], vtm[:, mt, ts(hB, HD)],
                                start=(m == 0), stop=(m == TPB - 1),
                                tile_position=(0, HD), skip_group_check=True)
                        t16 = tbp.tile([P, HD], F16, tag="t16",
                                       name=f"t16_{b}_{hp}")
                        nc.scalar.activation(t16[:], pt[:], AF.Copy)
                        t16s[(b, hp)] = t16

            attnT = bigp.tile([P, KT, T], F16, tag="big", name="attnT")
            with nc.named_scope("attn_m2"):
                for b in range(B_LOC):
                    for hp in range(KT):
                        t16 = t16s[(b, hp)]
                        for nw in range(2):
                            col = b * SEQ + nw * TW
                            ps = psA.tile([P, TW], F32, tag="psA")
                            nc.tensor.matmul(
                                ps[0:HD, :], t16[0:HD, :],
                                qkT[0:HD, hp, ds(col, TW)],
                                start=True, stop=True, tile_position=(0, 0))
                            nc.tensor.matmul(
                                ps[HD:P, :], t16[HD:P, :],
                                qkT[HD:P, hp, ds(col, TW)],
                                start=True, stop=True, tile_position=(HD, HD))
                            nc.vector.tensor_copy(attnT[:, hp, ds(col, TW)],
                                                  ps[:])

            # ------- gate + out projection (feature-major) -----------
            # gated^T accumulates feature-major; PE transposes bring each
            # token tile back and the residual add reads straight from
            # PSUM (XBAR is unreliable with tight consumer timing)
            gatedT = bigp.tile([P, KT, T], F16, tag="big", name="gatedT")
            with nc.named_scope("out"):
                for tw in range(NTW):
                    for j in range(KT):
                        psg = psA.tile([P, TW], F32, tag="psA")
                        for k in range(KT):
                            nc.tensor.matmul(
                                psg[:], wT["gate_w"][:, k, ts(j, P)],
                                xnT[:, k, ts(tw, TW)],
                                start=(k == 0), stop=(k == KT - 1))
                        g16 = gwk.tile([P, TW], F16, tag="g16")
                        nc.scalar.activation(g16[:], psg[:], AF.Sigmoid,
                                             bias=pbias[:, PB_GATE, j:j + 1])

                        pso = psA.tile([P, TW], F32, tag="psA")
                        for k in range(KT):
                            nc.tensor.matmul(
                                pso[:], wT["out_w"][:, k, ts(j, P)],
                                attnT[:, k, ts(tw, TW)],
                                start=(k == 0), stop=(k == KT - 1))
                        o16 = gwk.tile([P, TW], F16, tag="o16")
                        nc.scalar.activation(o16[:], pso[:], AF.Identity,
                                             bias=pbias[:, PB_OUT, j:j + 1])
                        nc.vector.tensor_mul(gatedT[:, j, ts(tw, TW)],
                                             g16[:], o16[:])
                    # back to token-major on PE + residual from PSUM;
                    # fp16 stores, 2 tiles per instruction
                    for half in range(RB // SB):
                        xo = xop.tile([P, SB, D], F16, tag="xo")
                        for r2 in range(SB):
                            i = tw * RB + half * SB + r2
                            pt = psT.tile([P, D], F16, tag="psT")
                            for k in range(KT):
                                nc.tensor.transpose(pt[:, ts(k, P)],
                                                    gatedT[:, k, ts(i, P)],
                                                    ident[:])
                            nc.vector.tensor_add(xo[:, r2, :], pt[:],
                                                 xs[:, i, :])
                        i0 = tw * RB + half * SB
                        nc.sync.dma_start(
                            out_t.ap()[:, ds(i0 * D, SB * D)], xo[:])

    nc.finalize()
    return nc


_NC = None


def _get_nc():
    global _NC
    if _NC is None:
        _NC = build_nc()
    return _NC


def make_in_maps(inputs, n_cores=8):
    f32 = np.float32
    x = np.asarray(inputs["x"], dtype=f32).astype(np.float16)
    ln_w = np.asarray(inputs["ln_w"], dtype=f32)
    ln_b = np.asarray(inputs["ln_b"], dtype=f32)

    # per-head output-feature permutation (evens then odds) makes the
    # on-device rope slices contiguous; pure layout prep
    perm = np.concatenate(
        [h * HD + np.concatenate([np.arange(0, HD, 2), np.arange(1, HD, 2)])
         for h in range(H)])

    shared = {}
    # weights: fold LN affine into enc/gate, transpose, cast fp16,
    # flatten to [p, k*D] so each partition line is one contiguous burst
    wmat = {nm: np.asarray(inputs[nm], dtype=f32) for nm in W_NAMES}
    wmat["enc_w"] = wmat["enc_w"] * ln_w[None, :]
    wmat["gate_w"] = wmat["gate_w"] * ln_w[None, :]
    wmat["qk_w"] = wmat["qk_w"][perm]
    for nm in W_NAMES:
        wt = wmat[nm].T.astype(np.float16)            # [d, j]
        wt = wt.reshape(KT, P, D).transpose(1, 0, 2)  # [p, k, j]
        shared[nm] = np.ascontiguousarray(wt.reshape(P, KT * D))

    enc_w = np.asarray(inputs["enc_w"], dtype=f32)
    gate_w = np.asarray(inputs["gate_w"], dtype=f32)
    encb = np.asarray(inputs["enc_b"], dtype=f32) + enc_w @ ln_b
    gate_b = np.asarray(inputs["gate_b"], dtype=f32) + gate_w @ ln_b
    out_b = np.asarray(inputs["out_b"], dtype=f32)
    pbias = np.stack([encb, out_b, gate_b]).reshape(3, KT, P)
    shared["pbias"] = np.ascontiguousarray(pbias.transpose(2, 0, 1))

    bvecs = np.stack([
        np.asarray(inputs["qk_b"], dtype=f32)[perm],
        np.asarray(inputs["v_b"], dtype=f32),
    ]).astype(np.float16)
    shared["bvecs"] = np.ascontiguousarray(
        np.broadcast_to(bvecs[None], (P, 2, D)))

    # rope tables A/C from rope_emb (host trig, fp16): per head block
    # A = [cosE | cosO], C = [sinO | -sinE]; o = x*A + swap(x*C).
    # Pre-scaled so the qk.qk^T product carries 1/sqrt(HD).
    ang = np.asarray(inputs["rope_emb"], dtype=np.float64)[:, :HD]
    cos, sin = np.cos(ang) * QK_SCALE, np.sin(ang) * QK_SCALE
    ahead = np.concatenate([cos[:, 0::2], cos[:, 1::2]], axis=1)  # [N, 64]
    chead = np.concatenate([sin[:, 1::2], -sin[:, 0::2]], axis=1)
    acfull = np.stack([np.tile(ahead, (1, H)), np.tile(chead, (1, H))],
                      axis=1)                        # [N, 2, D]
    acfull = acfull.reshape(TPB, P, 2, D).transpose(1, 2, 0, 3)
    shared["rope_ac"] = np.ascontiguousarray(acfull.astype(np.float16))

    in_maps = []
    for c in range(n_cores):
        m = dict(shared)
        # per-partition-contiguous x layout: [p, (tile d)]
        xc = x[c * B_LOC:(c + 1) * B_LOC].reshape(NT, P, D)
        m["x"] = np.ascontiguousarray(
            xc.transpose(1, 0, 2).reshape(P, NT * D))
        in_maps.append(m)
    return in_maps


def kernel(**inputs):
    nc = _get_nc()
    n_cores = 8
    in_maps = make_in_maps(inputs, n_cores)
    res = bass_utils.run_bass_kernel_spmd(
        nc, in_maps, core_ids=list(range(n_cores)))
    outs = []
    for r in res.results:
        o = np.asarray(r["out"]).reshape(P, NT, D).transpose(1, 0, 2)
        outs.append(o.reshape(B_LOC, SEQ, D).astype(np.float32))
    return np.concatenate(outs, axis=0)


# revision 23
# speedup vs baseline: 1.0642x; 1.0642x over previous
"""Trainium2 Bass kernel for nn_BDHBlock (pre-LN latent block with
softmax-free attention and sigmoid gating).

Sharding: data-parallel over batch B=16 across 8 cores (2 per core).
No collectives; outputs are gathered/re-laid-out on the host.

Per-core math (B_loc=2, N=1024, D=768, H=12, HD=64), all matmuls fp16
with fp32 PSUM accumulation:
  z    = (x - mu) * rstd                          (token-major)
  lat  = relu(z @ enc_w'.T + enc_b')              (feature-major)
  qk   = rope(lat @ qk_w.T + qk_b) / sqrt(sqrt(HD))   (token-major)
  v    = lat @ v_w.T + v_b                        (token-major)
  T_h  = qk_h^T @ v_h         per (b,h)           [HD, HD]
  attn_h = qk_h @ T_h      (== (qk qk^T/8) v by associativity)
  gated = sigmoid(z @ gate_w'.T + gate_b') * (attn @ out_w.T + out_b)
  out  = x + gated

enc_w' = enc_w*diag(ln_w), enc_b' = enc_b + enc_w@ln_b (same for gate)
fold the LayerNorm affine into the weights host-side.  DMA on this
part is latency-bound per instruction (~4us for 128 partition lines
regardless of bytes), so x ships fp16 in a per-partition-contiguous
host layout and loads in 3 instructions into a resident slab that
also serves the residual (no reload); the output stores fp16 in the
same layout, 2 tiles per instruction, and the host converts back.
Weights ride the gpsimd SWDGE ring.  LayerNorm emission is software-
pipelined (stats of tile i ahead of the apply of tile i-1) so the
in-order DVE queue never starves the per-tile scalar chain.  Rope is
o = x*A + swapped_halves(x*C) with host-precomputed fp16 tables (sign
folded into C), emitted one batch behind the qk drains so it never
blocks them.  xn^T and gated^T transposes run on PE; qk^T uses the
DMA XBAR (its consumers are far downstream).  Gate/out projections
are feature-major so biases ride the ACT drain (sigmoid fused), and
the residual add reads the PE-transposed gated result straight from
PSUM.  The softmax-free attention makes scores@v associative, so the
N x N score matrices are never materialized.
"""

import os
import sys

for _p in ("/opt/trn_rl_repo", "/root/.axon_site/_ro/trn_rl_repo"):
    if os.path.isdir(_p) and _p not in sys.path:
        sys.path.insert(0, _p)

import math
import numpy as np

import concourse.bass as bass
import concourse.mybir as mybir
from concourse import bacc
from concourse import bass_utils
from concourse.bass import ts, ds
from concourse.tile import TileContext
from concourse.masks import make_identity

F32 = mybir.dt.float32
F16 = mybir.dt.float16
AF = mybir.ActivationFunctionType

P = 128          # partitions
D = 768
KT = D // P      # 6 d-tiles
B_LOC = 2        # batch elements per core
SEQ = 1024
T = B_LOC * SEQ  # 2048 tokens per core
NT = T // P      # 16 token tiles
TPB = SEQ // P   # 8 token tiles per batch element
TW = 512         # token window (feature-major matmul free dim)
NTW = T // TW    # 4
JW = 384         # feature window (token-major matmul free dim)
NJW = D // JW    # 2
H = 12
HD = 64
EPS = 1e-5
QK_SCALE = 1.0 / math.sqrt(math.sqrt(HD))  # applied twice => 1/sqrt(HD)
RB = 4           # token tiles per rope batch / transpose window
SB = 2           # token tiles per output store

W_NAMES = ["enc_w", "qk_w", "v_w", "out_w", "gate_w"]
BV_QK, BV_V = 0, 1               # bvec rows (broadcast free-dim biases)
PB_ENC, PB_OUT, PB_GATE = 0, 1, 2  # pbias rows (per-partition biases)


def build_nc():
    nc = bacc.Bacc("TRN2", target_bir_lowering=False, debug=False)

    x_in = nc.dram_tensor("x", [P, NT * D], F16, kind="ExternalInput")
    ac_in = nc.dram_tensor("rope_ac", [P, 2, TPB, D], F16,
                           kind="ExternalInput")
    pbias_in = nc.dram_tensor("pbias", [P, 3, KT], F32, kind="ExternalInput")
    bvec_in = nc.dram_tensor("bvecs", [P, 2, D], F16, kind="ExternalInput")
    w_in = {nm: nc.dram_tensor(nm, [P, KT * D], F16, kind="ExternalInput")
            for nm in W_NAMES}
    out_t = nc.dram_tensor("out", [P, NT * D], F16, kind="ExternalOutput")

    with TileContext(nc) as tc:
        with (
            tc.tile_pool(name="consts", bufs=1) as cp,
            tc.tile_pool(name="wrot", bufs=3) as wrot,
            tc.tile_pool(name="big", bufs=4) as bigp,
            tc.tile_pool(name="xslab", bufs=1) as xsp,
            tc.tile_pool(name="work", bufs=2) as wk,
            tc.tile_pool(name="stats", bufs=4) as stp,
            tc.tile_pool(name="ropem2", bufs=1) as rm2,
            tc.tile_pool(name="qraw", bufs=2) as qrp,
            tc.tile_pool(name="gwk", bufs=2) as gwk,
            tc.tile_pool(name="xo", bufs=2) as xop,
            tc.tile_pool(name="tbuf", bufs=12) as tbp,
            tc.tile_pool(name="psA", bufs=3, space="PSUM") as psA,
            tc.tile_pool(name="psB", bufs=3, space="PSUM") as psB,
            tc.tile_pool(name="psT", bufs=2, space="PSUM") as psT,
        ):
            # ------------- constants / weights -----------------------
            # rope A/C tables take a big-pool ring slot; they are dead
            # after the last rope batch and vtm reuses the slot.
            ac = bigp.tile([P, 2, TPB, D], F16, tag="big", name="rope_ac")
            with nc.named_scope("prep"):
                eps_t = cp.tile([P, 1], F32, tag="epsc")
                nc.vector.memset(eps_t[:], EPS)
                ident = cp.tile([P, P], F16, tag="ident")
                make_identity(nc, ident[:])
                # per-partition biases for feature-major drains
                pbias = cp.tile([P, 3, KT], F32, tag="pbias")
                nc.gpsimd.dma_start(pbias[:], pbias_in.ap())

            xnT = cp.tile([P, KT, T], F16, tag="xnT")

            # x: resident slab, loaded in 3 latency-bound instructions
            # on the sync ring (first covers tiles 0-1 so LN starts asap)
            xs = xsp.tile([P, NT, D], F16, tag="xs")
            for lo, hi in ((0, 2), (2, 8), (8, NT)):
                nc.sync.dma_start(xs[:, lo:hi, :],
                                  x_in.ap()[:, ds(lo * D, (hi - lo) * D)])

            # weights / tables on the gpsimd SWDGE ring (own ring, and
            # DMA here is latency- not bandwidth-bound)
            wT = {}
            wT["enc_w"] = wrot.tile([P, KT, D], F16, tag="wT", name="wT_enc")
            nc.gpsimd.dma_start(wT["enc_w"][:], w_in["enc_w"].ap())
            bvec = cp.tile([P, 2, D], F16, tag="bvec")
            nc.gpsimd.dma_start(bvec[:], bvec_in.ap())
            nc.gpsimd.dma_start(ac[:], ac_in.ap())
            for nm in ["qk_w", "v_w"]:
                wT[nm] = wrot.tile([P, KT, D], F16, tag="wT", name=f"wT_{nm}")
                nc.gpsimd.dma_start(wT[nm][:], w_in[nm].ap())

            # ---------------- LayerNorm (token-major) ----------------
            # software-pipelined: stats of tile i are emitted ahead of
            # the apply of tile i-1 so the in-order DVE queue never
            # waits on a not-yet-landed tile before finishing an apply
            def ln_stats(i):
                xg = xs[:, i, :].rearrange("p (s c) -> p s c", c=256)
                stats = stp.tile([P, 3, 6], F32, tag="bnstats")
                for s in range(3):
                    nc.vector.bn_stats(stats[:, s, :], xg[:, s, :])
                mv = stp.tile([P, 2], F32, tag="bnmv")
                nc.vector.bn_aggr(mv[:], stats[:])
                return mv

            def ln_apply(i, mv):
                rs = stp.tile([P, 1], F32, tag="rstd")
                nc.scalar.activation(rs[:], mv[:, 1:2], AF.Sqrt,
                                     bias=eps_t[:])
                nc.vector.reciprocal(rs[:], rs[:])
                nb = stp.tile([P, 1], F32, tag="negmurs")
                nc.vector.tensor_scalar(
                    nb[:], mv[:, 0:1], rs[:], -1.0,
                    op0=mybir.AluOpType.mult, op1=mybir.AluOpType.mult)
                xn16 = wk.tile([P, D], F16, tag="xn16")
                nc.scalar.activation(xn16[:], xs[:, i, :], AF.Identity,
                                     bias=nb[:], scale=rs[:])
                # feature-major via PE transposes (PE is idle here),
                # batched into one PSUM bank + one DVE drain
                pt = psT.tile([P, D], F16, tag="psT")
                for k in range(KT):
                    nc.tensor.transpose(pt[:, ts(k, P)], xn16[:, ts(k, P)],
                                        ident[:])
                nc.vector.tensor_copy(
                    xnT[:, :, ts(i, P)],
                    pt[:].rearrange("p (k c) -> p k c", c=P))

            with nc.named_scope("ln"):
                mvs = {}
                mvs[0] = ln_stats(0)
                for i in range(NT):
                    if i + 1 < NT:
                        mvs[i + 1] = ln_stats(i + 1)
                    ln_apply(i, mvs.pop(i))

            # ---------------- encoder: latT = relu(Wenc @ xn^T) ------
            latT = bigp.tile([P, KT, T], F16, tag="big", name="latT")
            with nc.named_scope("enc"):
                for tw in range(NTW):
                    for j in range(KT):
                        ps = psA.tile([P, TW], F32, tag="psA")
                        for k in range(KT):
                            nc.tensor.matmul(
                                ps[:], wT["enc_w"][:, k, ts(j, P)],
                                xnT[:, k, ts(tw, TW)],
                                start=(k == 0), stop=(k == KT - 1))
                        nc.scalar.activation(latT[:, j, ts(tw, TW)], ps[:],
                                             AF.Relu,
                                             bias=pbias[:, PB_ENC, j:j + 1])

            # late weights reuse the first two wrot slots (deps auto-wait)
            for nm in ["out_w", "gate_w"]:
                wT[nm] = wrot.tile([P, KT, D], F16, tag="wT", name=f"wT_{nm}")
                nc.gpsimd.dma_start(wT[nm][:], w_in[nm].ap())

            # ---------------- qk (token-major) + rope ----------------
            qkR = bigp.tile([P, NT, D], F16, tag="big", name="qkR")
            qkT = bigp.tile([P, KT, T], F16, tag="big", name="qkT")
            qraws = {}

            def qk_mm(g):
                qraw = qrp.tile([P, RB, D], F16, tag="qraw")
                qraws[g] = qraw
                for r in range(RB):
                    i = g * RB + r
                    for jw in range(NJW):
                        ps = psB.tile([P, JW], F32, tag="psB")
                        for k in range(KT):
                            nc.tensor.matmul(
                                ps[:], latT[:, k, ts(i, P)],
                                wT["qk_w"][:, k, ts(jw, JW)],
                                start=(k == 0), stop=(k == KT - 1))
                        nc.vector.tensor_add(
                            qraw[:, r, ts(jw, JW)], ps[:],
                            bvec[:, BV_QK, ts(jw, JW)])

            def rope_muls(g, m2_engine):
                # o = x*A + swap_halves(x*C); m2 first (m1 is in-place)
                qraw = qraws[g]
                ti0 = (g * RB) % TPB
                aA = ac[:, 0, ds(ti0, RB), :]
                aC = ac[:, 1, ds(ti0, RB), :]
                m2 = rm2.tile([P, RB, D], F16, tag="ropem2")
                m2_engine.tensor_mul(m2[:], qraw[:], aC)
                nc.vector.tensor_mul(qraw[:], qraw[:], aA)
                return m2

            def rope_adds(g, m2):
                # adds recombine halves across m1/m2
                qraw = qraws.pop(g)
                m1h = qraw[:].rearrange("p t (f d) -> p t f d", d=HD)
                m2h = m2[:].rearrange("p t (f d) -> p t f d", d=HD)
                oh = qkR[:, ds(g * RB, RB), :].rearrange(
                    "p t (f d) -> p t f d", d=HD)
                nc.vector.tensor_add(
                    oh[:, :, :, 0:HD // 2],
                    m1h[:, :, :, 0:HD // 2], m2h[:, :, :, HD // 2:])
                nc.vector.tensor_add(
                    oh[:, :, :, HD // 2:],
                    m1h[:, :, :, HD // 2:], m2h[:, :, :, 0:HD // 2])
                # feature-major copy via DMA XBAR transpose
                for r in range(RB):
                    i = g * RB + r
                    nc.sync.dma_start(qkT[:, :, ts(i, P)], qkR[:, i, :],
                                      transpose=True)

            def rope(g, m2_engine=None):
                rope_adds(g, rope_muls(g, m2_engine or nc.vector))

            with nc.named_scope("qk"):
                for g in range(NT // RB):
                    qk_mm(g)
                    if g >= 1:
                        rope(g - 1)  # one batch behind: never blocks drains

            # ---------------- v (token-major) ------------------------
            vtm = bigp.tile([P, NT, D], F16, tag="big", name="v")
            with nc.named_scope("v"):
                for i in range(NT):
                    for jw in range(NJW):
                        ps = psB.tile([P, JW], F32, tag="psB")
                        for k in range(KT):
                            nc.tensor.matmul(
                                ps[:], latT[:, k, ts(i, P)],
                                wT["v_w"][:, k, ts(jw, JW)],
                                start=(k == 0), stop=(k == KT - 1))
                        nc.vector.tensor_add(vtm[:, i, ts(jw, JW)], ps[:],
                                             bvec[:, BV_V, ts(jw, JW)])
                    if i == 1:
                        rope(NT // RB - 1)

            # ---------------- attention ------------------------------
            # M1: T_h = qk_h^T @ v_h  [HD, HD] per (b, head); head pairs
            # packed into array column halves.  M2: attnT_h = T_h^T @ qkT_h.
            t16s = {}
            with nc.named_scope("attn_m1"):
                for b in range(B_LOC):
                    for hp in range(KT):
                        hA, hB = 2 * hp, 2 * hp + 1
                        # the two concurrent accum groups live on disjoint
                        # partition ranges / array quadrants; the sim's
                        # bank-granular group check is stricter than HW
                        pt = psB.tile([P, HD], F32, tag="psB",
                                      name=f"ptm1_{b}_{hp}")
                        for m in range(TPB):
                            mt = b * TPB + m
                            nc.tensor.matmul(
                                pt[0:HD, :],
                                qkR[:, mt, ts(hA, HD)], vtm[:, mt, ts(hA, HD)],
                                start=(m == 0), stop=(m == TPB - 1),
                                tile_position=(0, 0), skip_group_check=True)
                            nc.tensor.matmul(
                                pt[HD:P, :],
                                qkR[:, mt, ts(hB, HD)], vtm[:, mt, ts(hB, HD)],
                                start=(m == 0), stop=(m == TPB - 1),
                                tile_position=(0, HD), skip_group_check=True)
                        t16 = tbp.tile([P, HD], F16, tag="t16",
                                       name=f"t16_{b}_{hp}")
                        nc.scalar.activation(t16[:], pt[:], AF.Copy)
                        t16s[(b, hp)] = t16

            attnT = bigp.tile([P, KT, T], F16, tag="big", name="attnT")
            with nc.named_scope("attn_m2"):
                for b in range(B_LOC):
                    for hp in range(KT):
                        t16 = t16s[(b, hp)]
                        for nw in range(2):
                            col = b * SEQ + nw * TW
                            ps = psA.tile([P, TW], F32, tag="psA")
                            nc.tensor.matmul(
                                ps[0:HD, :], t16[0:HD, :],
                                qkT[0:HD, hp, ds(col, TW)],
                                start=True, stop=True, tile_position=(0, 0))
                            nc.tensor.matmul(
                                ps[HD:P, :], t16[HD:P, :],
                                qkT[HD:P, hp, ds(col, TW)],
                                start=True, stop=True, tile_position=(HD, HD))
                            nc.scalar.activation(attnT[:, hp, ds(col, TW)],
                                                 ps[:], AF.Copy)

            # ------- gate + out projection (feature-major) -----------
            # gated^T accumulates feature-major; PE transposes bring each
            # token tile back and the residual add reads straight from
            # PSUM (XBAR is unreliable with tight consumer timing)
            gatedT = bigp.tile([P, KT, T], F16, tag="big", name="gatedT")
            with nc.named_scope("out"):
                for tw in range(NTW):
                    for j in range(KT):
                        psg = psA.tile([P, TW], F32, tag="psA")
                        for k in range(KT):
                            nc.tensor.matmul(
                                psg[:], wT["gate_w"][:, k, ts(j, P)],
                                xnT[:, k, ts(tw, TW)],
                                start=(k == 0), stop=(k == KT - 1))
                        g16 = gwk.tile([P, TW], F16, tag="g16")
                        nc.scalar.activation(g16[:], psg[:], AF.Sigmoid,
                                             bias=pbias[:, PB_GATE, j:j + 1])

                        pso = psA.tile([P, TW], F32, tag="psA")
                        for k in range(KT):
                            nc.tensor.matmul(
                                pso[:], wT["out_w"][:, k, ts(j, P)],
                                attnT[:, k, ts(tw, TW)],
                                start=(k == 0), stop=(k == KT - 1))
                        o16 = gwk.tile([P, TW], F16, tag="o16")
                        nc.scalar.activation(o16[:], pso[:], AF.Identity,
                                             bias=pbias[:, PB_OUT, j:j + 1])
                        nc.vector.tensor_mul(gatedT[:, j, ts(tw, TW)],
                                             g16[:], o16[:])
                    # back to token-major on PE + residual from PSUM;
                    # fp16 stores, 2 tiles per instruction
                    for half in range(RB // SB):
                        xo = xop.tile([P, SB, D], F16, tag="xo")
                        for r2 in range(SB):
                            i = tw * RB + half * SB + r2
                            pt = psT.tile([P, D], F16, tag="psT")
                            for k in range(KT):
                                nc.tensor.transpose(pt[:, ts(k, P)],
                                                    gatedT[:, k, ts(i, P)],
                                                    ident[:])
                            nc.vector.tensor_add(xo[:, r2, :], pt[:],
                                                 xs[:, i, :])
                        i0 = tw * RB + half * SB
                        nc.sync.dma_start(
                            out_t.ap()[:, ds(i0 * D, SB * D)], xo[:])

    nc.finalize()
    return nc


_NC = None


def _get_nc():
    global _NC
    if _NC is None:
        _NC = build_nc()
    return _NC


def make_in_maps(inputs, n_cores=8):
    f32 = np.float32
    x = np.asarray(inputs["x"], dtype=f32).astype(np.float16)
    ln_w = np.asarray(inputs["ln_w"], dtype=f32)
    ln_b = np.asarray(inputs["ln_b"], dtype=f32)

    # per-head output-feature permutation (evens then odds) makes the
    # on-device rope slices contiguous; pure layout prep
    perm = np.concatenate(
        [h * HD + np.concatenate([np.arange(0, HD, 2), np.arange(1, HD, 2)])
         for h in range(H)])

    shared = {}
    # weights: fold LN affine into enc/gate, transpose, cast fp16,
    # flatten to [p, k*D] so each partition line is one contiguous burst
    wmat = {nm: np.asarray(inputs[nm], dtype=f32) for nm in W_NAMES}
    wmat["enc_w"] = wmat["enc_w"] * ln_w[None, :]
    wmat["gate_w"] = wmat["gate_w"] * ln_w[None, :]
    wmat["qk_w"] = wmat["qk_w"][perm]
    for nm in W_NAMES:
        wt = wmat[nm].T.astype(np.float16)            # [d, j]
        wt = wt.reshape(KT, P, D).transpose(1, 0, 2)  # [p, k, j]
        shared[nm] = np.ascontiguousarray(wt.reshape(P, KT * D))

    enc_w = np.asarray(inputs["enc_w"], dtype=f32)
    gate_w = np.asarray(inputs["gate_w"], dtype=f32)
    encb = np.asarray(inputs["enc_b"], dtype=f32) + enc_w @ ln_b
    gate_b = np.asarray(inputs["gate_b"], dtype=f32) + gate_w @ ln_b
    out_b = np.asarray(inputs["out_b"], dtype=f32)
    pbias = np.stack([encb, out_b, gate_b]).reshape(3, KT, P)
    shared["pbias"] = np.ascontiguousarray(pbias.transpose(2, 0, 1))

    bvecs = np.stack([
        np.asarray(inputs["qk_b"], dtype=f32)[perm],
        np.asarray(inputs["v_b"], dtype=f32),
    ]).astype(np.float16)
    shared["bvecs"] = np.ascontiguousarray(
        np.broadcast_to(bvecs[None], (P, 2, D)))

    # rope tables A/C from rope_emb (host trig, fp16): per head block
    # A = [cosE | cosO], C = [sinO | -sinE]; o = x*A + swap(x*C).
    # Pre-scaled so the qk.qk^T product carries 1/sqrt(HD).
    ang = np.asarray(inputs["rope_emb"], dtype=np.float64)[:, :HD]
    cos, sin = np.cos(ang) * QK_SCALE, np.sin(ang) * QK_SCALE
    ahead = np.concatenate([cos[:, 0::2], cos[:, 1::2]], axis=1)  # [N, 64]
    chead = np.concatenate([sin[:, 1::2], -sin[:, 0::2]], axis=1)
    acfull = np.stack([np.tile(ahead, (1, H)), np.tile(chead, (1, H))],
                      axis=1)                        # [N, 2, D]
    acfull = acfull.reshape(TPB, P, 2, D).transpose(1, 2, 0, 3)
    shared["rope_ac"] = np.ascontiguousarray(acfull.astype(np.float16))

    in_maps = []
    for c in range(n_cores):
        m = dict(shared)
        # per-partition-contiguous x layout: [p, (tile d)]
        xc = x[c * B_LOC:(c + 1) * B_LOC].reshape(NT, P, D)
        m["x"] = np.ascontiguousarray(
            xc.transpose(1, 0, 2).reshape(P, NT * D))
        in_maps.append(m)
    return in_maps


def kernel(**inputs):
    nc = _get_nc()
    n_cores = 8
    in_maps = make_in_maps(inputs, n_cores)
    res = bass_utils.run_bass_kernel_spmd(
        nc, in_maps, core_ids=list(range(n_cores)))
    outs = []
    for r in res.results:
        o = np.asarray(r["out"]).reshape(P, NT, D).transpose(1, 0, 2)
        outs.append(o.reshape(B_LOC, SEQ, D).astype(np.float32))
    return np.concatenate(outs, axis=0)
